# revision 1
# baseline (speedup 1.0000x reference)
"""CausalSelfAttention (depthwise-conv + RoPE + causal SDPA + proj) on 8 Trainium2 cores.

Tensor-parallel over heads: each core computes 2 of 16 heads end-to-end plus its
partial output projection; the host sums the 8 partial projections.
All matmuls run in float32r. Layouts are transposed ([dim, time]) so DMA is
contiguous and softmax denominators come from a PE ones-matmul."""
import sys
sys.path.insert(0, '/opt/trn_rl_repo')
import numpy as np
import concourse.bass as bass
import concourse.mybir as mybir
import concourse.tile as tile
from concourse import bacc
from concourse import bass_utils

F32 = mybir.dt.float32
F32R = mybir.dt.float32r
AF = mybir.ActivationFunctionType
OP = mybir.AluOpType

B, T, C = 2, 2048, 2048
H, D = 16, 128
NC = 8
HPC = H // NC          # heads per core = 2
CHW = 512              # chunk width (t)
NCH = T // CHW         # chunks per batch = 4
CT = C // 128          # 16 c-tiles
STW = 128              # s-tile width


def build_program():
    nc = bacc.Bacc("TRN2", target_bir_lowering=False, debug=False, num_devices=NC)

    xT = nc.dram_tensor("xT", [C, B * T], F32, kind="ExternalInput").ap()
    wq = nc.dram_tensor("wq", [C, HPC * D], F32, kind="ExternalInput").ap()
    wk = nc.dram_tensor("wk", [C, HPC * D], F32, kind="ExternalInput").ap()
    wv = nc.dram_tensor("wv", [C, HPC * D], F32, kind="ExternalInput").ap()
    wo = nc.dram_tensor("wo", [HPC * D, C], F32, kind="ExternalInput").ap()
    cosT_d = nc.dram_tensor("cosT", [D, T], F32, kind="ExternalInput").ap()
    sinS_d = nc.dram_tensor("sinS", [D, T], F32, kind="ExternalInput").ap()
    masks_d = nc.dram_tensor("masks", [STW, 4 * CHW], F32, kind="ExternalInput").ap()
    convw_d = nc.dram_tensor("convw", [C, 3], F32, kind="ExternalInput").ap()
    ones_d = nc.dram_tensor("ones", [128, 128], F32, kind="ExternalInput").ap()
    ident_d = nc.dram_tensor("ident", [128, 128], F32, kind="ExternalInput").ap()
    outT = nc.dram_tensor("outT", [C, B * T], F32, kind="ExternalOutput").ap()

    with tile.TileContext(nc) as tc:
        with (
            tc.tile_pool(name="wr", bufs=1) as wr,
            tc.tile_pool(name="const", bufs=1) as cst,
            tc.tile_pool(name="wstg", bufs=4) as wstg,
            tc.tile_pool(name="xt", bufs=2) as xtp,
            tc.tile_pool(name="xc", bufs=1) as xcp,
            tc.tile_pool(name="tmp", bufs=2) as tmp,
            tc.tile_pool(name="rd", bufs=2) as rdp,
            tc.tile_pool(name="qs", bufs=1) as qsp,
            tc.tile_pool(name="kv", bufs=1) as kvp,
            tc.tile_pool(name="vstg", bufs=2) as vstgp,
            tc.tile_pool(name="e", bufs=3) as ep,
            tc.tile_pool(name="y", bufs=1) as yp,
            tc.tile_pool(name="pmm", bufs=2, space="PSUM") as pmm,
            tc.tile_pool(name="pS", bufs=2, space="PSUM") as pS,
            tc.tile_pool(name="pU", bufs=2, space="PSUM") as pU,
            tc.tile_pool(name="pD", bufs=2, space="PSUM") as pD,
        ):
            # ---- constants ----
            cosT = cst.tile([D, T], F32, tag="cosT")
            nc.sync.dma_start(cosT[:], cosT_d[:])
            sinS = cst.tile([D, T], F32, tag="sinS")
            nc.sync.dma_start(sinS[:], sinS_d[:])
            masks = cst.tile([STW, 4 * CHW], F32, tag="masks")
            nc.sync.dma_start(masks[:], masks_d[:])
            ident = cst.tile([128, 128], F32, tag="ident")
            nc.sync.dma_start(ident[:], ident_d[:])
            cw = []
            for ct in range(CT):
                t_ = cst.tile([128, 3], F32, tag=f"cw{ct}")
                nc.sync.dma_start(t_[:], convw_d[ct * 128:(ct + 1) * 128, :])
                cw.append(t_)

            ones_st = wstg.tile([128, 128], F32, tag="wst_on")
            nc.sync.dma_start(ones_st[:], ones_d[:])
            ones_r = wr.tile([128, 128], F32R, tag="ones_r")
            nc.vector.tensor_copy(ones_r[:], ones_st[:])

            # ---- weights -> f32r resident tiles ----
            wq_r, wk_r, wv_r = [], [], []
            for name, dsrc, dst in (("q", wq, wq_r), ("k", wk, wk_r), ("v", wv, wv_r)):
                for ct in range(CT):
                    stg = wstg.tile([128, HPC * D], F32, tag="wst")
                    nc.sync.dma_start(stg[:], dsrc[ct * 128:(ct + 1) * 128, :])
                    t_ = wr.tile([128, HPC * D], F32R, tag=f"w{name}r{ct}")
                    nc.vector.tensor_copy(t_[:], stg[:])
                    dst.append(t_)
            wo_r = []
            for hi in range(HPC):
                t_ = wr.tile([128, C], F32R, tag=f"wor{hi}")
                for j in range(C // 256):
                    stg = wstg.tile([128, 256], F32, tag="wst_o")
                    nc.sync.dma_start(stg[:], wo[hi * 128:(hi + 1) * 128, j * 256:(j + 1) * 256])
                    nc.vector.tensor_copy(t_[:, j * 256:(j + 1) * 256], stg[:])
                wo_r.append(t_)

            # ---- main loop ----
            for b in range(B):
                k_all = [kvp.tile([D, T], F32R, tag=f"k{h}", name=f"kall{b}_{h}") for h in range(HPC)]
                v_all = [kvp.tile([128, T], F32R, tag=f"v{h}", name=f"vall{b}_{h}") for h in range(HPC)]
                for ch in range(NCH):
                    g0 = b * T + ch * CHW          # global col offset into xT/outT
                    t0 = ch * CHW                  # within-batch t offset
                    # ---- conv: xc[ct] = depthwise causal conv ----
                    xc = []
                    for ct in range(CT):
                        xt = xtp.tile([128, CHW + 2], F32, tag="xt")
                        if ch == 0:
                            nc.gpsimd.memset(xt[:, 0:2], 0.0)
                            nc.sync.dma_start(xt[:, 2:CHW + 2], xT[ct * 128:(ct + 1) * 128, g0:g0 + CHW])
                        else:
                            nc.sync.dma_start(xt[:], xT[ct * 128:(ct + 1) * 128, g0 - 2:g0 + CHW])
                        ta = tmp.tile([128, CHW], F32, tag="t1")
                        nc.scalar.mul(ta[:], xt[:, 0:CHW], cw[ct][:, 0:1])
                        tb = tmp.tile([128, CHW], F32, tag="t2")
                        nc.vector.scalar_tensor_tensor(tb[:], xt[:, 1:CHW + 1], cw[ct][:, 1:2], ta[:], OP.mult, OP.add)
                        xct = xcp.tile([128, CHW], F32R, tag=f"xc{ct}")
                        nc.vector.scalar_tensor_tensor(xct[:], xt[:, 2:CHW + 2], cw[ct][:, 2:3], tb[:], OP.mult, OP.add)
                        xc.append(xct)

                    # ---- QKV + rope ----
                    q_sb = []
                    for h in range(HPC):
                        hs = slice(h * D, (h + 1) * D)
                        # q
                        q_ps = pmm.tile([128, CHW], F32, tag="mm")
                        for ct in range(CT):
                            nc.tensor.matmul(q_ps[:], wq_r[ct][:, hs], xc[ct][:],
                                             start=(ct == 0), stop=(ct == CT - 1))
                        a = tmp.tile([128, CHW], F32, tag="t1")
                        nc.vector.tensor_tensor(a[:], q_ps[:], cosT[:, t0:t0 + CHW], OP.mult)
                        bb = tmp.tile([128, CHW], F32, tag="t2")
                        nc.vector.tensor_tensor(bb[0:64, :], q_ps[64:128, :], sinS[0:64, t0:t0 + CHW], OP.mult)
                        nc.vector.tensor_tensor(bb[64:128, :], q_ps[0:64, :], sinS[64:128, t0:t0 + CHW], OP.mult)
                        qt = qsp.tile([128, CHW], F32R, tag=f"q{h}")
                        nc.vector.tensor_tensor(qt[:], a[:], bb[:], OP.add)
                        q_sb.append(qt)
                        # k
                        k_ps = pmm.tile([128, CHW], F32, tag="mm")
                        for ct in range(CT):
                            nc.tensor.matmul(k_ps[:], wk_r[ct][:, hs], xc[ct][:],
                                             start=(ct == 0), stop=(ct == CT - 1))
                        a2 = tmp.tile([128, CHW], F32, tag="t1")
                        nc.vector.tensor_tensor(a2[:], k_ps[:], cosT[:, t0:t0 + CHW], OP.mult)
                        b2 = tmp.tile([128, CHW], F32, tag="t2")
                        nc.vector.tensor_tensor(b2[0:64, :], k_ps[64:128, :], sinS[0:64, t0:t0 + CHW], OP.mult)
                        nc.vector.tensor_tensor(b2[64:128, :], k_ps[0:64, :], sinS[64:128, t0:t0 + CHW], OP.mult)
                        nc.vector.tensor_tensor(k_all[h][:, t0:t0 + CHW], a2[:], b2[:], OP.add)
                        # v
                        v_ps = pmm.tile([128, CHW], F32, tag="mm")
                        for ct in range(CT):
                            nc.tensor.matmul(v_ps[:], wv_r[ct][:, hs], xc[ct][:],
                                             start=(ct == 0), stop=(ct == CT - 1))
                        vstg = vstgp.tile([128, CHW], F32, tag="vstg")
                        nc.scalar.copy(vstg[:], v_ps[:])
                        for j in range(CHW // 128):
                            tp = pS.tile([128, 128], F32, tag="S")
                            nc.tensor.transpose(tp[:], vstg[:, j * 128:(j + 1) * 128], ident[:])
                            srow = t0 + j * 128
                            nc.vector.tensor_copy(v_all[h][:, srow:srow + 128], tp[:])

                    # ---- attention ----
                    yT = []
                    n_st = 4 * ch + 4
                    for h in range(HPC):
                        U_ps = pU.tile([128, CHW], F32, tag="U")
                        D_ps = pD.tile([128, CHW], F32, tag="Dn")
                        for st in range(n_st):
                            s_ps = pS.tile([128, CHW], F32, tag="S")
                            nc.tensor.matmul(s_ps[:], k_all[h][:, st * STW:(st + 1) * STW], q_sb[h][:],
                                             start=True, stop=True)
                            e = ep.tile([128, CHW], F32R, tag="e")
                            nc.scalar.activation(e[:], s_ps[:], AF.Exp)
                            if st >= 4 * ch:
                                i = st - 4 * ch
                                nc.vector.tensor_tensor(e[:], e[:], masks[:, i * CHW:(i + 1) * CHW], OP.mult)
                            nc.tensor.matmul(U_ps[:], v_all[h][:, st * STW:(st + 1) * STW], e[:],
                                             start=(st == 0), stop=(st == n_st - 1))
                            nc.tensor.matmul(D_ps[:], ones_r[:], e[:],
                                             start=(st == 0), stop=(st == n_st - 1))
                        rD = rdp.tile([128, CHW], F32, tag="rd")
                        nc.vector.reciprocal(rD[:], D_ps[:])
                        yt = yp.tile([128, CHW], F32R, tag=f"y{h}")
                        nc.vector.tensor_tensor(yt[:], U_ps[:], rD[:], OP.mult)
                        yT.append(yt)

                    # ---- proj (partial over this core's heads) ----
                    for oc in range(CT):
                        o_ps = pmm.tile([128, CHW], F32, tag="mm")
                        nc.tensor.matmul(o_ps[:], wo_r[0][:, oc * 128:(oc + 1) * 128], yT[0][:],
                                         start=True, stop=False)
                        nc.tensor.matmul(o_ps[:], wo_r[1][:, oc * 128:(oc + 1) * 128], yT[1][:],
                                         start=False, stop=True)
                        o_sb = vstgp.tile([128, CHW], F32, tag="osb")
                        nc.scalar.copy(o_sb[:], o_ps[:])
                        nc.sync.dma_start(outT[oc * 128:(oc + 1) * 128, g0:g0 + CHW], o_sb[:])

    nc.compile()
    return nc


def host_prepare(x, conv_w, w_attn, w_proj):
    """Build per-core input maps."""
    xT = np.ascontiguousarray(x.transpose(2, 0, 1).reshape(C, B * T)).astype(np.float32)
    convw = np.ascontiguousarray(conv_w[:, 0, :]).astype(np.float32)

    t = np.arange(T, dtype=np.float64)
    inv_freq = 1.0 / (10000.0 ** (np.arange(0, D, 2, dtype=np.float64) / D))
    freqs = np.outer(t, inv_freq)                      # [T, 64]
    emb = np.concatenate([freqs, freqs], axis=1)       # [T, 128]
    cosT = np.cos(emb).T.astype(np.float32)            # [128, T]
    sinT = np.sin(emb).T.astype(np.float32)
    sinS = sinT.copy()
    sinS[0:64] = -sinT[0:64]

    masks = np.zeros((STW, 4 * CHW), dtype=np.float32)
    s_idx = np.arange(STW)[:, None]
    t_idx = np.arange(CHW)[None, :]
    for i in range(4):
        masks[:, i * CHW:(i + 1) * CHW] = (i * STW + s_idx <= t_idx).astype(np.float32)

    ones = np.ones((128, 128), dtype=np.float32)
    ident = np.eye(128, dtype=np.float32)

    scale = 1.0 / np.sqrt(np.float32(D))
    in_maps = []
    for c in range(NC):
        h0 = c * HPC
        rq = slice(h0 * D, (h0 + HPC) * D)
        wq_c = np.ascontiguousarray((w_attn[rq.start:rq.stop, :] * scale).T).astype(np.float32)
        wk_c = np.ascontiguousarray(w_attn[C + rq.start:C + rq.stop, :].T).astype(np.float32)
        wv_c = np.ascontiguousarray(w_attn[2 * C + rq.start:2 * C + rq.stop, :].T).astype(np.float32)
        wo_c = np.ascontiguousarray(w_proj[:, rq.start:rq.stop].T).astype(np.float32)
        in_maps.append({
            "xT": xT, "wq": wq_c, "wk": wk_c, "wv": wv_c, "wo": wo_c,
            "cosT": cosT, "sinS": sinS, "masks": masks, "convw": convw,
            "ones": ones, "ident": ident,
        })
    return in_maps


def host_finish(results):
    acc = np.zeros((C, B * T), dtype=np.float64)
    for r in results:
        acc += r["outT"]
    return acc.reshape(C, B, T).transpose(1, 2, 0).astype(np.float32)


_CACHE = {}




def kernel(x, conv_w, w_attn, w_proj):
    x = np.ascontiguousarray(x, dtype=np.float32)
    conv_w = np.ascontiguousarray(conv_w, dtype=np.float32)
    w_attn = np.ascontiguousarray(w_attn, dtype=np.float32)
    w_proj = np.ascontiguousarray(w_proj, dtype=np.float32)
    if "nc" not in _CACHE:
        _CACHE["nc"] = build_program()
    in_maps = host_prepare(x, conv_w, w_attn, w_proj)
    res = bass_utils.run_bass_kernel_spmd(_CACHE["nc"], in_maps, core_ids=list(range(NC)))
    return host_finish(res.results)



# revision 6
# speedup vs baseline: 1.5505x; 1.5505x over previous
"""CausalSelfAttention (depthwise-conv + RoPE + causal SDPA + proj) on 8 Trainium2 cores.

Tensor-parallel over heads: each core computes 2 of 16 heads end-to-end plus its
partial output projection. Per-call I/O is minimized: x arrives as a per-core
1/8 channel shard in bf16 and is AllGathered on device; the 8 partial output
projections are summed with an on-device ReduceScatter so each core emits only
its 1/8 row-slice of the output. All matmuls run in bf16 (f32 PSUM accum)."""
import sys
sys.path.insert(0, '/opt/trn_rl_repo')
import numpy as np
import concourse.bass as bass
import concourse.mybir as mybir
import concourse.tile as tile
from concourse import bacc
from concourse import bass_utils

F32 = mybir.dt.float32
BF16 = mybir.dt.bfloat16
NPBF16 = mybir.dt.np(BF16)
AF = mybir.ActivationFunctionType
OP = mybir.AluOpType

B, T, C = 2, 2048, 2048
H, D = 16, 128
NC = 8
HPC = H // NC          # heads per core = 2
CSH = C // NC          # channel-shard rows per core = 256
CHW = 512              # chunk width (t)
NCH = T // CHW         # chunks per batch = 4
CT = C // 128          # 16 c-tiles
STW = 128              # s-tile width


def build_program():
    nc = bacc.Bacc("TRN2", target_bir_lowering=False, debug=False, num_devices=NC)

    xs_d = nc.dram_tensor("xs", [CSH, B * T], BF16, kind="ExternalInput").ap()
    wq = nc.dram_tensor("wq", [C, HPC * D], BF16, kind="ExternalInput").ap()
    wk = nc.dram_tensor("wk", [C, HPC * D], BF16, kind="ExternalInput").ap()
    wv = nc.dram_tensor("wv", [C, HPC * D], BF16, kind="ExternalInput").ap()
    wo = nc.dram_tensor("wo", [HPC * D, C], BF16, kind="ExternalInput").ap()
    cosT_d = nc.dram_tensor("cosT", [D, T], F32, kind="ExternalInput").ap()
    sinS_d = nc.dram_tensor("sinS", [D, T], F32, kind="ExternalInput").ap()
    masks_d = nc.dram_tensor("masks", [STW, 4 * CHW], BF16, kind="ExternalInput").ap()
    convw_d = nc.dram_tensor("convw", [C, 3], F32, kind="ExternalInput").ap()
    ones_d = nc.dram_tensor("ones", [128, 128], BF16, kind="ExternalInput").ap()
    outS = nc.dram_tensor("outS", [CSH, B * T], F32, kind="ExternalOutput").ap()

    groups = [list(range(NC))]

    from contextlib import ExitStack
    with tile.TileContext(nc) as tc:
        with ExitStack() as stack:
            ent = stack.enter_context
            dram = ent(tc.tile_pool(name="dram", bufs=1, space="DRAM"))
            wr = ent(tc.tile_pool(name="wr", bufs=1))
            cst = ent(tc.tile_pool(name="const", bufs=1))
            xtp = ent(tc.tile_pool(name="xt", bufs=2))
            xcp = ent(tc.tile_pool(name="xc", bufs=1))
            tmp = ent(tc.tile_pool(name="tmp", bufs=2))
            rdp = ent(tc.tile_pool(name="rd", bufs=2))
            qsp = ent(tc.tile_pool(name="qs", bufs=1))
            kvp = ent(tc.tile_pool(name="kv", bufs=1))
            vstgp = ent(tc.tile_pool(name="vstg", bufs=2))
            ep = ent(tc.tile_pool(name="e", bufs=3))
            yp = ent(tc.tile_pool(name="y", bufs=1))
            pmm = ent(tc.tile_pool(name="pmm", bufs=2, space="PSUM"))
            pS = ent(tc.tile_pool(name="pS", bufs=2, space="PSUM"))
            pU = ent(tc.tile_pool(name="pU", bufs=2, space="PSUM"))
            pD = ent(tc.tile_pool(name="pD", bufs=1, space="PSUM"))
            pV = ent(tc.tile_pool(name="pV", bufs=1, space="PSUM"))
            # ---- x shard -> AllGather into full [C, B*T] bf16 in DRAM ----
            xs_b = dram.tile([CSH, B * T], BF16)
            xg = dram.tile([C, B * T], BF16)
            nc.gpsimd.dma_start(xs_b[:], xs_d[:])
            nc.gpsimd.collective_compute(
                "AllGather", mybir.AluOpType.bypass,
                replica_groups=groups, ins=[xs_b.opt()], outs=[xg.opt()])
            # partial-projection accumulator + ReduceScatter output
            oacc = dram.tile([C, B * T], F32)
            ors = dram.tile([CSH, B * T], F32)

            # ---- constants ----
            cosT = cst.tile([D, T], F32, tag="cosT")
            nc.sync.dma_start(cosT[:], cosT_d[:])
            sinS = cst.tile([D, T], F32, tag="sinS")
            nc.sync.dma_start(sinS[:], sinS_d[:])
            masks = cst.tile([STW, 4 * CHW], BF16, tag="masks")
            nc.sync.dma_start(masks[:], masks_d[:])
            ones_r = cst.tile([128, 128], BF16, tag="ones")
            nc.sync.dma_start(ones_r[:], ones_d[:])
            cw = []
            for ct in range(CT):
                t_ = cst.tile([128, 3], F32, tag=f"cw{ct}")
                nc.sync.dma_start(t_[:], convw_d[ct * 128:(ct + 1) * 128, :])
                cw.append(t_)

            # ---- weights -> resident bf16 tiles ----
            wq_r, wk_r, wv_r = [], [], []
            for name, dsrc, dst in (("q", wq, wq_r), ("k", wk, wk_r), ("v", wv, wv_r)):
                for ct in range(CT):
                    t_ = wr.tile([128, HPC * D], BF16, tag=f"w{name}r{ct}")
                    nc.sync.dma_start(t_[:], dsrc[ct * 128:(ct + 1) * 128, :])
                    dst.append(t_)
            wo_r = []
            for hi in range(HPC):
                t_ = wr.tile([128, C], BF16, tag=f"wor{hi}")
                nc.sync.dma_start(t_[:], wo[hi * 128:(hi + 1) * 128, :])
                wo_r.append(t_)

            # ---- main loop ----
            for b in range(B):
                k_all = [kvp.tile([D, T], BF16, tag=f"k{h}", name=f"kall{b}_{h}") for h in range(HPC)]
                v_all = [kvp.tile([128, T], BF16, tag=f"v{h}", name=f"vall{b}_{h}") for h in range(HPC)]
                for ch in range(NCH):
                    g0 = b * T + ch * CHW          # global col offset into xg/oacc
                    t0 = ch * CHW                  # within-batch t offset
                    # ---- conv: xc[ct] = depthwise causal conv ----
                    xc = []
                    for ct in range(CT):
                        xt = xtp.tile([128, CHW + 2], BF16, tag="xt")
                        if ch == 0:
                            nc.gpsimd.memset(xt[:, 0:2], 0.0)
                            nc.sync.dma_start(xt[:, 2:CHW + 2], xg[ct * 128:(ct + 1) * 128, g0:g0 + CHW])
                        else:
                            nc.sync.dma_start(xt[:], xg[ct * 128:(ct + 1) * 128, g0 - 2:g0 + CHW])
                        ta = tmp.tile([128, CHW], F32, tag="t1")
                        nc.scalar.mul(ta[:], xt[:, 0:CHW], cw[ct][:, 0:1])
                        tb = tmp.tile([128, CHW], F32, tag="t2")
                        nc.vector.scalar_tensor_tensor(tb[:], xt[:, 1:CHW + 1], cw[ct][:, 1:2], ta[:], OP.mult, OP.add)
                        xct = xcp.tile([128, CHW], BF16, tag=f"xc{ct}")
                        nc.vector.scalar_tensor_tensor(xct[:], xt[:, 2:CHW + 2], cw[ct][:, 2:3], tb[:], OP.mult, OP.add)
                        xc.append(xct)

                    # ---- QKV + rope ----
                    q_sb = []
                    for h in range(HPC):
                        hs = slice(h * D, (h + 1) * D)
                        # q
                        q_ps = pmm.tile([128, CHW], F32, tag="mm")
                        for ct in range(CT):
                            nc.tensor.matmul(q_ps[:], wq_r[ct][:, hs], xc[ct][:],
                                             start=(ct == 0), stop=(ct == CT - 1))
                        a = tmp.tile([128, CHW], F32, tag="t1")
                        nc.vector.tensor_tensor(a[:], q_ps[:], cosT[:, t0:t0 + CHW], OP.mult)
                        bb = tmp.tile([128, CHW], F32, tag="t2")
                        nc.vector.tensor_tensor(bb[0:64, :], q_ps[64:128, :], sinS[0:64, t0:t0 + CHW], OP.mult)
                        nc.vector.tensor_tensor(bb[64:128, :], q_ps[0:64, :], sinS[64:128, t0:t0 + CHW], OP.mult)
                        qt = qsp.tile([128, CHW], BF16, tag=f"q{h}")
                        nc.vector.tensor_tensor(qt[:], a[:], bb[:], OP.add)
                        q_sb.append(qt)
                        # k
                        k_ps = pmm.tile([128, CHW], F32, tag="mm")
                        for ct in range(CT):
                            nc.tensor.matmul(k_ps[:], wk_r[ct][:, hs], xc[ct][:],
                                             start=(ct == 0), stop=(ct == CT - 1))
                        a2 = tmp.tile([128, CHW], F32, tag="t1")
                        nc.vector.tensor_tensor(a2[:], k_ps[:], cosT[:, t0:t0 + CHW], OP.mult)
                        b2 = tmp.tile([128, CHW], F32, tag="t2")
                        nc.vector.tensor_tensor(b2[0:64, :], k_ps[64:128, :], sinS[0:64, t0:t0 + CHW], OP.mult)
                        nc.vector.tensor_tensor(b2[64:128, :], k_ps[0:64, :], sinS[64:128, t0:t0 + CHW], OP.mult)
                        nc.vector.tensor_tensor(k_all[h][:, t0:t0 + CHW], a2[:], b2[:], OP.add)
                    # ---- v: computed directly in [s, d] block layout (both heads) ----
                    for j in range(CHW // 128):
                        v_ps = pV.tile([128, HPC * D], F32, tag="vmm")
                        for ct in range(CT):
                            nc.tensor.matmul(v_ps[:], xc[ct][:, j * 128:(j + 1) * 128], wv_r[ct][:],
                                             start=(ct == 0), stop=(ct == CT - 1))
                        srow = t0 + j * 128
                        for h in range(HPC):
                            nc.vector.tensor_copy(v_all[h][:, srow:srow + 128], v_ps[:, h * D:(h + 1) * D])

                    # ---- attention ----
                    yT = []
                    n_st = 4 * ch + 4
                    for h in range(HPC):
                        U_ps = pU.tile([128, CHW], F32, tag="U")
                        D_ps = pD.tile([128, CHW], F32, tag="Dn")
                        for st in range(n_st):
                            s_ps = pS.tile([128, CHW], F32, tag="S")
                            nc.tensor.matmul(s_ps[:], k_all[h][:, st * STW:(st + 1) * STW], q_sb[h][:],
                                             start=True, stop=True)
                            e = ep.tile([128, CHW], BF16, tag="e")
                            nc.scalar.activation(e[:], s_ps[:], AF.Exp)
                            if st >= 4 * ch:
                                i = st - 4 * ch
                                nc.vector.tensor_tensor(e[:], e[:], masks[:, i * CHW:(i + 1) * CHW], OP.mult)
                            nc.tensor.matmul(U_ps[:], v_all[h][:, st * STW:(st + 1) * STW], e[:],
                                             start=(st == 0), stop=(st == n_st - 1))
                            nc.tensor.matmul(D_ps[:], ones_r[:], e[:],
                                             start=(st == 0), stop=(st == n_st - 1))
                        rD = rdp.tile([128, CHW], F32, tag="rd")
                        nc.vector.reciprocal(rD[:], D_ps[:])
                        yt = yp.tile([128, CHW], BF16, tag=f"y{h}")
                        nc.vector.tensor_tensor(yt[:], U_ps[:], rD[:], OP.mult)
                        yT.append(yt)

                    # ---- proj (partial over this core's heads) ----
                    for oc in range(CT):
                        o_ps = pmm.tile([128, CHW], F32, tag="mm")
                        nc.tensor.matmul(o_ps[:], wo_r[0][:, oc * 128:(oc + 1) * 128], yT[0][:],
                                         start=True, stop=False)
                        nc.tensor.matmul(o_ps[:], wo_r[1][:, oc * 128:(oc + 1) * 128], yT[1][:],
                                         start=False, stop=True)
                        o_sb = vstgp.tile([128, CHW], F32, tag="osb")
                        nc.scalar.copy(o_sb[:], o_ps[:])
                        nc.sync.dma_start(oacc[oc * 128:(oc + 1) * 128, g0:g0 + CHW], o_sb[:])

            # ---- sum the 8 partial projections; each core keeps its row-slice ----
            nc.gpsimd.collective_compute(
                "ReduceScatter", mybir.AluOpType.add,
                replica_groups=groups, ins=[oacc.opt()], outs=[ors.opt()])
            nc.gpsimd.dma_start(outS[:], ors[:])

    nc.compile()
    return nc


def host_prepare(x, conv_w, w_attn, w_proj):
    """Build per-core input maps."""
    xT = np.ascontiguousarray(x.transpose(2, 0, 1).reshape(C, B * T)).astype(NPBF16)
    convw = np.ascontiguousarray(conv_w[:, 0, :]).astype(np.float32)

    t = np.arange(T, dtype=np.float64)
    inv_freq = 1.0 / (10000.0 ** (np.arange(0, D, 2, dtype=np.float64) / D))
    freqs = np.outer(t, inv_freq)                      # [T, 64]
    emb = np.concatenate([freqs, freqs], axis=1)       # [T, 128]
    cosT = np.cos(emb).T.astype(np.float32)            # [128, T]
    sinT = np.sin(emb).T.astype(np.float32)
    sinS = sinT.copy()
    sinS[0:64] = -sinT[0:64]

    masks = np.zeros((STW, 4 * CHW), dtype=np.float32)
    s_idx = np.arange(STW)[:, None]
    t_idx = np.arange(CHW)[None, :]
    for i in range(4):
        masks[:, i * CHW:(i + 1) * CHW] = (i * STW + s_idx <= t_idx).astype(np.float32)
    masks = masks.astype(NPBF16)

    ones = np.ones((128, 128), dtype=NPBF16)

    scale = 1.0 / np.sqrt(np.float32(D))
    in_maps = []
    for c in range(NC):
        h0 = c * HPC
        rq = slice(h0 * D, (h0 + HPC) * D)
        wq_c = np.ascontiguousarray((w_attn[rq.start:rq.stop, :] * scale).T).astype(NPBF16)
        wk_c = np.ascontiguousarray(w_attn[C + rq.start:C + rq.stop, :].T).astype(NPBF16)
        wv_c = np.ascontiguousarray(w_attn[2 * C + rq.start:2 * C + rq.stop, :].T).astype(NPBF16)
        wo_c = np.ascontiguousarray(w_proj[:, rq.start:rq.stop].T).astype(NPBF16)
        xs_c = np.ascontiguousarray(xT[c * CSH:(c + 1) * CSH, :])
        in_maps.append({
            "xs": xs_c, "wq": wq_c, "wk": wk_c, "wv": wv_c, "wo": wo_c,
            "cosT": cosT, "sinS": sinS, "masks": masks, "convw": convw,
            "ones": ones,
        })
    return in_maps


def host_finish(results):
    full = np.concatenate([r["outS"] for r in results], axis=0)   # [C, B*T]
    return full.reshape(C, B, T).transpose(1, 2, 0).astype(np.float32)


_CACHE = {}


def kernel(x, conv_w, w_attn, w_proj):
    x = np.ascontiguousarray(x, dtype=np.float32)
    conv_w = np.ascontiguousarray(conv_w, dtype=np.float32)
    w_attn = np.ascontiguousarray(w_attn, dtype=np.float32)
    w_proj = np.ascontiguousarray(w_proj, dtype=np.float32)
    if "nc" not in _CACHE:
        _CACHE["nc"] = build_program()
    in_maps = host_prepare(x, conv_w, w_attn, w_proj)
    res = bass_utils.run_bass_kernel_spmd(_CACHE["nc"], in_maps, core_ids=list(range(NC)))
    return host_finish(res.results)


# revision 7
# speedup vs baseline: 1.6044x; 1.0347x over previous
"""CausalSelfAttention (depthwise-conv + RoPE + causal SDPA + proj) on 8 Trainium2 cores.

Tensor-parallel over heads: each core computes 2 of 16 heads end-to-end plus its
partial output projection. Per-call I/O is minimized: x arrives as a per-core
1/8 channel shard in bf16 and is AllGathered on device; the 8 partial output
projections are summed with an on-device ReduceScatter so each core emits only
its 1/8 row-slice of the output. All matmuls run in bf16 (f32 PSUM accum)."""
import sys
sys.path.insert(0, '/opt/trn_rl_repo')
import numpy as np
import concourse.bass as bass
import concourse.mybir as mybir
import concourse.tile as tile
from concourse import bacc
from concourse import bass_utils

F32 = mybir.dt.float32
BF16 = mybir.dt.bfloat16
NPBF16 = mybir.dt.np(BF16)
AF = mybir.ActivationFunctionType
OP = mybir.AluOpType

B, T, C = 2, 2048, 2048
H, D = 16, 128
NC = 8
HPC = H // NC          # heads per core = 2
CSH = C // NC          # channel-shard rows per core = 256
CHW = 512              # chunk width (t)
NCH = T // CHW         # chunks per batch = 4
CT = C // 128          # 16 c-tiles
STW = 128              # s-tile width


def build_program():
    nc = bacc.Bacc("TRN2", target_bir_lowering=False, debug=False, num_devices=NC)

    xs_d = nc.dram_tensor("xs", [CSH, B * T], BF16, kind="ExternalInput").ap()
    wq = nc.dram_tensor("wq", [C, HPC * D], BF16, kind="ExternalInput").ap()
    wk = nc.dram_tensor("wk", [C, HPC * D], BF16, kind="ExternalInput").ap()
    wv = nc.dram_tensor("wv", [C, HPC * D], BF16, kind="ExternalInput").ap()
    wo = nc.dram_tensor("wo", [HPC * D, C], BF16, kind="ExternalInput").ap()
    cosT_d = nc.dram_tensor("cosT", [D, T], BF16, kind="ExternalInput").ap()
    sinS_d = nc.dram_tensor("sinS", [D, T], BF16, kind="ExternalInput").ap()
    masks_d = nc.dram_tensor("masks", [STW, 4 * CHW], BF16, kind="ExternalInput").ap()
    convw_d = nc.dram_tensor("convw", [C, 3], F32, kind="ExternalInput").ap()
    ones_d = nc.dram_tensor("ones", [128, 128], BF16, kind="ExternalInput").ap()
    outS = nc.dram_tensor("outS", [CSH, B * T], BF16, kind="ExternalOutput").ap()

    groups = [list(range(NC))]

    from contextlib import ExitStack
    with tile.TileContext(nc) as tc:
        with ExitStack() as stack:
            ent = stack.enter_context
            dram = ent(tc.tile_pool(name="dram", bufs=1, space="DRAM"))
            wr = ent(tc.tile_pool(name="wr", bufs=1))
            cst = ent(tc.tile_pool(name="const", bufs=1))
            xtp = ent(tc.tile_pool(name="xt", bufs=2))
            xcp = ent(tc.tile_pool(name="xc", bufs=1))
            tmp = ent(tc.tile_pool(name="tmp", bufs=2))
            rdp = ent(tc.tile_pool(name="rd", bufs=2))
            qsp = ent(tc.tile_pool(name="qs", bufs=1))
            kvp = ent(tc.tile_pool(name="kv", bufs=1))
            vstgp = ent(tc.tile_pool(name="vstg", bufs=2))
            ep = ent(tc.tile_pool(name="e", bufs=3))
            yp = ent(tc.tile_pool(name="y", bufs=1))
            pmm = ent(tc.tile_pool(name="pmm", bufs=2, space="PSUM"))
            pS = ent(tc.tile_pool(name="pS", bufs=2, space="PSUM"))
            pU = ent(tc.tile_pool(name="pU", bufs=2, space="PSUM"))
            pD = ent(tc.tile_pool(name="pD", bufs=1, space="PSUM"))
            pV = ent(tc.tile_pool(name="pV", bufs=1, space="PSUM"))
            # ---- x shard -> AllGather into full [C, B*T] bf16 in DRAM ----
            xs_b = dram.tile([CSH, B * T], BF16)
            xg = dram.tile([C, B * T], BF16)
            nc.gpsimd.dma_start(xs_b[:], xs_d[:])
            nc.gpsimd.collective_compute(
                "AllGather", mybir.AluOpType.bypass,
                replica_groups=groups, ins=[xs_b.opt()], outs=[xg.opt()])
            # partial-projection accumulator + ReduceScatter output
            oacc = dram.tile([C, B * T], BF16)
            ors = dram.tile([CSH, B * T], BF16)

            # ---- constants ----
            cosT = cst.tile([D, T], BF16, tag="cosT")
            nc.sync.dma_start(cosT[:], cosT_d[:])
            sinS = cst.tile([D, T], BF16, tag="sinS")
            nc.sync.dma_start(sinS[:], sinS_d[:])
            masks = cst.tile([STW, 4 * CHW], BF16, tag="masks")
            nc.sync.dma_start(masks[:], masks_d[:])
            ones_r = cst.tile([128, 128], BF16, tag="ones")
            nc.sync.dma_start(ones_r[:], ones_d[:])
            cw = []
            for ct in range(CT):
                t_ = cst.tile([128, 3], F32, tag=f"cw{ct}")
                nc.sync.dma_start(t_[:], convw_d[ct * 128:(ct + 1) * 128, :])
                cw.append(t_)

            # ---- weights -> resident bf16 tiles ----
            wq_r, wk_r, wv_r = [], [], []
            for name, dsrc, dst in (("q", wq, wq_r), ("k", wk, wk_r), ("v", wv, wv_r)):
                for ct in range(CT):
                    t_ = wr.tile([128, HPC * D], BF16, tag=f"w{name}r{ct}")
                    nc.sync.dma_start(t_[:], dsrc[ct * 128:(ct + 1) * 128, :])
                    dst.append(t_)
            wo_r = []
            for hi in range(HPC):
                t_ = wr.tile([128, C], BF16, tag=f"wor{hi}")
                nc.sync.dma_start(t_[:], wo[hi * 128:(hi + 1) * 128, :])
                wo_r.append(t_)

            # ---- main loop ----
            for b in range(B):
                k_all = [kvp.tile([D, T], BF16, tag=f"k{h}", name=f"kall{b}_{h}") for h in range(HPC)]
                v_all = [kvp.tile([128, T], BF16, tag=f"v{h}", name=f"vall{b}_{h}") for h in range(HPC)]
                for ch in range(NCH):
                    g0 = b * T + ch * CHW          # global col offset into xg/oacc
                    t0 = ch * CHW                  # within-batch t offset
                    # ---- conv: xc[ct] = depthwise causal conv ----
                    xc = []
                    for ct in range(CT):
                        xt = xtp.tile([128, CHW + 2], BF16, tag="xt")
                        if ch == 0:
                            nc.gpsimd.memset(xt[:, 0:2], 0.0)
                            nc.sync.dma_start(xt[:, 2:CHW + 2], xg[ct * 128:(ct + 1) * 128, g0:g0 + CHW])
                        else:
                            nc.sync.dma_start(xt[:], xg[ct * 128:(ct + 1) * 128, g0 - 2:g0 + CHW])
                        ta = tmp.tile([128, CHW], F32, tag="t1")
                        nc.scalar.mul(ta[:], xt[:, 0:CHW], cw[ct][:, 0:1])
                        tb = tmp.tile([128, CHW], F32, tag="t2")
                        nc.vector.scalar_tensor_tensor(tb[:], xt[:, 1:CHW + 1], cw[ct][:, 1:2], ta[:], OP.mult, OP.add)
                        xct = xcp.tile([128, CHW], BF16, tag=f"xc{ct}")
                        nc.vector.scalar_tensor_tensor(xct[:], xt[:, 2:CHW + 2], cw[ct][:, 2:3], tb[:], OP.mult, OP.add)
                        xc.append(xct)

                    # ---- QKV + rope ----
                    q_sb = []
                    for h in range(HPC):
                        hs = slice(h * D, (h + 1) * D)
                        # q
                        q_ps = pmm.tile([128, CHW], F32, tag="mm")
                        for ct in range(CT):
                            nc.tensor.matmul(q_ps[:], wq_r[ct][:, hs], xc[ct][:],
                                             start=(ct == 0), stop=(ct == CT - 1))
                        a = tmp.tile([128, CHW], F32, tag="t1")
                        nc.vector.tensor_tensor(a[:], q_ps[:], cosT[:, t0:t0 + CHW], OP.mult)
                        bb = tmp.tile([128, CHW], F32, tag="t2")
                        nc.vector.tensor_tensor(bb[0:64, :], q_ps[64:128, :], sinS[0:64, t0:t0 + CHW], OP.mult)
                        nc.vector.tensor_tensor(bb[64:128, :], q_ps[0:64, :], sinS[64:128, t0:t0 + CHW], OP.mult)
                        qt = qsp.tile([128, CHW], BF16, tag=f"q{h}")
                        nc.vector.tensor_tensor(qt[:], a[:], bb[:], OP.add)
                        q_sb.append(qt)
                        # k
                        k_ps = pmm.tile([128, CHW], F32, tag="mm")
                        for ct in range(CT):
                            nc.tensor.matmul(k_ps[:], wk_r[ct][:, hs], xc[ct][:],
                                             start=(ct == 0), stop=(ct == CT - 1))
                        a2 = tmp.tile([128, CHW], F32, tag="t1")
                        nc.vector.tensor_tensor(a2[:], k_ps[:], cosT[:, t0:t0 + CHW], OP.mult)
                        b2 = tmp.tile([128, CHW], F32, tag="t2")
                        nc.vector.tensor_tensor(b2[0:64, :], k_ps[64:128, :], sinS[0:64, t0:t0 + CHW], OP.mult)
                        nc.vector.tensor_tensor(b2[64:128, :], k_ps[0:64, :], sinS[64:128, t0:t0 + CHW], OP.mult)
                        nc.vector.tensor_tensor(k_all[h][:, t0:t0 + CHW], a2[:], b2[:], OP.add)
                    # ---- v: computed directly in [s, d] block layout (both heads) ----
                    for j in range(CHW // 128):
                        v_ps = pV.tile([128, HPC * D], F32, tag="vmm")
                        for ct in range(CT):
                            nc.tensor.matmul(v_ps[:], xc[ct][:, j * 128:(j + 1) * 128], wv_r[ct][:],
                                             start=(ct == 0), stop=(ct == CT - 1))
                        srow = t0 + j * 128
                        for h in range(HPC):
                            nc.vector.tensor_copy(v_all[h][:, srow:srow + 128], v_ps[:, h * D:(h + 1) * D])

                    # ---- attention ----
                    yT = []
                    n_st = 4 * ch + 4
                    for h in range(HPC):
                        U_ps = pU.tile([128, CHW], F32, tag="U")
                        D_ps = pD.tile([128, CHW], F32, tag="Dn")
                        for st in range(n_st):
                            s_ps = pS.tile([128, CHW], F32, tag="S")
                            nc.tensor.matmul(s_ps[:], k_all[h][:, st * STW:(st + 1) * STW], q_sb[h][:],
                                             start=True, stop=True)
                            e = ep.tile([128, CHW], BF16, tag="e")
                            nc.scalar.activation(e[:], s_ps[:], AF.Exp)
                            if st >= 4 * ch:
                                i = st - 4 * ch
                                nc.vector.tensor_tensor(e[:], e[:], masks[:, i * CHW:(i + 1) * CHW], OP.mult)
                            nc.tensor.matmul(U_ps[:], v_all[h][:, st * STW:(st + 1) * STW], e[:],
                                             start=(st == 0), stop=(st == n_st - 1))
                            nc.tensor.matmul(D_ps[:], ones_r[:], e[:],
                                             start=(st == 0), stop=(st == n_st - 1))
                        rD = rdp.tile([128, CHW], F32, tag="rd")
                        nc.vector.reciprocal(rD[:], D_ps[:])
                        yt = yp.tile([128, CHW], BF16, tag=f"y{h}")
                        nc.vector.tensor_tensor(yt[:], U_ps[:], rD[:], OP.mult)
                        yT.append(yt)

                    # ---- proj (partial over this core's heads) ----
                    for oc in range(CT):
                        o_ps = pmm.tile([128, CHW], F32, tag="mm")
                        nc.tensor.matmul(o_ps[:], wo_r[0][:, oc * 128:(oc + 1) * 128], yT[0][:],
                                         start=True, stop=False)
                        nc.tensor.matmul(o_ps[:], wo_r[1][:, oc * 128:(oc + 1) * 128], yT[1][:],
                                         start=False, stop=True)
                        o_sb = vstgp.tile([128, CHW], BF16, tag="osb")
                        nc.scalar.copy(o_sb[:], o_ps[:])
                        nc.sync.dma_start(oacc[oc * 128:(oc + 1) * 128, g0:g0 + CHW], o_sb[:])

            # ---- sum the 8 partial projections; each core keeps its row-slice ----
            nc.gpsimd.collective_compute(
                "ReduceScatter", mybir.AluOpType.add,
                replica_groups=groups, ins=[oacc.opt()], outs=[ors.opt()])
            nc.gpsimd.dma_start(outS[:], ors[:])

    nc.compile()
    return nc


def host_prepare(x, conv_w, w_attn, w_proj):
    """Build per-core input maps."""
    xT = np.ascontiguousarray(x.transpose(2, 0, 1).reshape(C, B * T)).astype(NPBF16)
    convw = np.ascontiguousarray(conv_w[:, 0, :]).astype(np.float32)

    t = np.arange(T, dtype=np.float64)
    inv_freq = 1.0 / (10000.0 ** (np.arange(0, D, 2, dtype=np.float64) / D))
    freqs = np.outer(t, inv_freq)                      # [T, 64]
    emb = np.concatenate([freqs, freqs], axis=1)       # [T, 128]
    cosT = np.cos(emb).T.astype(np.float32).astype(NPBF16)  # [128, T]
    sinT = np.sin(emb).T.astype(np.float32)
    sinS = sinT.copy()
    sinS[0:64] = -sinT[0:64]
    sinS = sinS.astype(NPBF16)

    masks = np.zeros((STW, 4 * CHW), dtype=np.float32)
    s_idx = np.arange(STW)[:, None]
    t_idx = np.arange(CHW)[None, :]
    for i in range(4):
        masks[:, i * CHW:(i + 1) * CHW] = (i * STW + s_idx <= t_idx).astype(np.float32)
    masks = masks.astype(NPBF16)

    ones = np.ones((128, 128), dtype=NPBF16)

    scale = 1.0 / np.sqrt(np.float32(D))
    in_maps = []
    for c in range(NC):
        h0 = c * HPC
        rq = slice(h0 * D, (h0 + HPC) * D)
        wq_c = np.ascontiguousarray((w_attn[rq.start:rq.stop, :] * scale).T).astype(NPBF16)
        wk_c = np.ascontiguousarray(w_attn[C + rq.start:C + rq.stop, :].T).astype(NPBF16)
        wv_c = np.ascontiguousarray(w_attn[2 * C + rq.start:2 * C + rq.stop, :].T).astype(NPBF16)
        wo_c = np.ascontiguousarray(w_proj[:, rq.start:rq.stop].T).astype(NPBF16)
        xs_c = np.ascontiguousarray(xT[c * CSH:(c + 1) * CSH, :])
        in_maps.append({
            "xs": xs_c, "wq": wq_c, "wk": wk_c, "wv": wv_c, "wo": wo_c,
            "cosT": cosT, "sinS": sinS, "masks": masks, "convw": convw,
            "ones": ones,
        })
    return in_maps


def host_finish(results):
    full = np.concatenate([np.asarray(r["outS"], dtype=np.float32) for r in results], axis=0)   # [C, B*T]
    return full.reshape(C, B, T).transpose(1, 2, 0).astype(np.float32)


_CACHE = {}


def kernel(x, conv_w, w_attn, w_proj):
    x = np.ascontiguousarray(x, dtype=np.float32)
    conv_w = np.ascontiguousarray(conv_w, dtype=np.float32)
    w_attn = np.ascontiguousarray(w_attn, dtype=np.float32)
    w_proj = np.ascontiguousarray(w_proj, dtype=np.float32)
    if "nc" not in _CACHE:
        _CACHE["nc"] = build_program()
    in_maps = host_prepare(x, conv_w, w_attn, w_proj)
    res = bass_utils.run_bass_kernel_spmd(_CACHE["nc"], in_maps, core_ids=list(range(NC)))
    return host_finish(res.results)


# revision 8
# speedup vs baseline: 1.6329x; 1.0178x over previous
"""CausalSelfAttention (depthwise-conv + RoPE + causal SDPA + proj) on 8 Trainium2 cores.

Tensor-parallel over heads: each core computes 2 of 16 heads end-to-end plus its
partial output projection. Per-call I/O is minimized: x arrives as a per-core
1/8 channel shard in bf16 and is AllGathered on device; the 8 partial output
projections are summed with an on-device ReduceScatter so each core emits only
its 1/8 row-slice of the output. All matmuls run in bf16 (f32 PSUM accum)."""
import sys
sys.path.insert(0, '/opt/trn_rl_repo')
import numpy as np
import concourse.bass as bass
import concourse.mybir as mybir
import concourse.tile as tile
from concourse import bacc
from concourse import bass_utils

F32 = mybir.dt.float32
BF16 = mybir.dt.bfloat16
NPBF16 = mybir.dt.np(BF16)
AF = mybir.ActivationFunctionType
OP = mybir.AluOpType

B, T, C = 2, 2048, 2048
H, D = 16, 128
NC = 8
HPC = H // NC          # heads per core = 2
CSH = C // NC          # channel-shard rows per core = 256
CHW = 512              # chunk width (t)
NCH = T // CHW         # chunks per batch = 4
CT = C // 128          # 16 c-tiles
STW = 128              # s-tile width


def _const_tables():
    t = np.arange(T, dtype=np.float64)
    inv_freq = 1.0 / (10000.0 ** (np.arange(0, D, 2, dtype=np.float64) / D))
    freqs = np.outer(t, inv_freq)                      # [T, 64]
    emb = np.concatenate([freqs, freqs], axis=1)       # [T, 128]
    cosT = np.cos(emb).T.astype(np.float32).astype(NPBF16)  # [128, T]
    sinT = np.sin(emb).T.astype(np.float32)
    sinS = sinT.copy()
    sinS[0:64] = -sinT[0:64]
    sinS = sinS.astype(NPBF16)
    masks = np.zeros((STW, 4 * CHW), dtype=np.float32)
    s_idx = np.arange(STW)[:, None]
    t_idx = np.arange(CHW)[None, :]
    for i in range(4):
        masks[:, i * CHW:(i + 1) * CHW] = (i * STW + s_idx <= t_idx).astype(np.float32)
    masks = masks.astype(NPBF16)
    ones = np.ones((128, 128), dtype=NPBF16)
    return cosT, sinS, masks, ones


def build_program():
    nc = bacc.Bacc("TRN2", target_bir_lowering=False, debug=False, num_devices=NC)

    xs_d = nc.dram_tensor("xs", [CSH, B * T], BF16, kind="ExternalInput").ap()
    wq = nc.dram_tensor("wq", [C, HPC * D], BF16, kind="ExternalInput").ap()
    wk = nc.dram_tensor("wk", [C, HPC * D], BF16, kind="ExternalInput").ap()
    wv = nc.dram_tensor("wv", [C, HPC * D], BF16, kind="ExternalInput").ap()
    wo = nc.dram_tensor("wo", [HPC * D, C], BF16, kind="ExternalInput").ap()
    convw_d = nc.dram_tensor("convw", [C, 3], F32, kind="ExternalInput").ap()
    cosT_np, sinS_np, masks_np, ones_np = _const_tables()
    cosT_d = nc.inline_tensor(cosT_np, name="cosTc").ap()
    sinS_d = nc.inline_tensor(sinS_np, name="sinSc").ap()
    masks_d = nc.inline_tensor(masks_np, name="masksc").ap()
    ones_d = nc.inline_tensor(ones_np, name="onesc").ap()
    outS = nc.dram_tensor("outS", [CSH, B * T], BF16, kind="ExternalOutput").ap()

    groups = [list(range(NC))]

    from contextlib import ExitStack
    with tile.TileContext(nc) as tc:
        with ExitStack() as stack:
            ent = stack.enter_context
            dram = ent(tc.tile_pool(name="dram", bufs=1, space="DRAM"))
            wr = ent(tc.tile_pool(name="wr", bufs=1))
            cst = ent(tc.tile_pool(name="const", bufs=1))
            xtp = ent(tc.tile_pool(name="xt", bufs=2))
            xcp = ent(tc.tile_pool(name="xc", bufs=1))
            tmp = ent(tc.tile_pool(name="tmp", bufs=2))
            rdp = ent(tc.tile_pool(name="rd", bufs=2))
            qsp = ent(tc.tile_pool(name="qs", bufs=1))
            kvp = ent(tc.tile_pool(name="kv", bufs=1))
            vstgp = ent(tc.tile_pool(name="vstg", bufs=2))
            ep = ent(tc.tile_pool(name="e", bufs=3))
            yp = ent(tc.tile_pool(name="y", bufs=1))
            pmm = ent(tc.tile_pool(name="pmm", bufs=2, space="PSUM"))
            pS = ent(tc.tile_pool(name="pS", bufs=2, space="PSUM"))
            pU = ent(tc.tile_pool(name="pU", bufs=2, space="PSUM"))
            pD = ent(tc.tile_pool(name="pD", bufs=1, space="PSUM"))
            pV = ent(tc.tile_pool(name="pV", bufs=1, space="PSUM"))
            # ---- x shard -> AllGather into full [C, B*T] bf16 in DRAM ----
            xs_b = dram.tile([CSH, B * T], BF16)
            xg = dram.tile([C, B * T], BF16)
            nc.gpsimd.dma_start(xs_b[:], xs_d[:])
            nc.gpsimd.collective_compute(
                "AllGather", mybir.AluOpType.bypass,
                replica_groups=groups, ins=[xs_b.opt()], outs=[xg.opt()])
            # partial-projection accumulator + ReduceScatter output
            oacc = dram.tile([C, B * T], BF16)
            ors = dram.tile([CSH, B * T], BF16)

            # ---- constants ----
            cosT = cst.tile([D, T], BF16, tag="cosT")
            nc.sync.dma_start(cosT[:], cosT_d[:])
            sinS = cst.tile([D, T], BF16, tag="sinS")
            nc.sync.dma_start(sinS[:], sinS_d[:])
            masks = cst.tile([STW, 4 * CHW], BF16, tag="masks")
            nc.sync.dma_start(masks[:], masks_d[:])
            ones_r = cst.tile([128, 128], BF16, tag="ones")
            nc.sync.dma_start(ones_r[:], ones_d[:])
            cw = []
            for ct in range(CT):
                t_ = cst.tile([128, 3], F32, tag=f"cw{ct}")
                nc.sync.dma_start(t_[:], convw_d[ct * 128:(ct + 1) * 128, :])
                cw.append(t_)

            # ---- weights -> resident bf16 tiles ----
            wq_r, wk_r, wv_r = [], [], []
            for name, dsrc, dst in (("q", wq, wq_r), ("k", wk, wk_r), ("v", wv, wv_r)):
                for ct in range(CT):
                    t_ = wr.tile([128, HPC * D], BF16, tag=f"w{name}r{ct}")
                    nc.sync.dma_start(t_[:], dsrc[ct * 128:(ct + 1) * 128, :])
                    dst.append(t_)
            wo_r = []
            for hi in range(HPC):
                t_ = wr.tile([128, C], BF16, tag=f"wor{hi}")
                nc.sync.dma_start(t_[:], wo[hi * 128:(hi + 1) * 128, :])
                wo_r.append(t_)

            # ---- main loop ----
            for b in range(B):
                k_all = [kvp.tile([D, T], BF16, tag=f"k{h}", name=f"kall{b}_{h}") for h in range(HPC)]
                v_all = [kvp.tile([128, T], BF16, tag=f"v{h}", name=f"vall{b}_{h}") for h in range(HPC)]
                for ch in range(NCH):
                    g0 = b * T + ch * CHW          # global col offset into xg/oacc
                    t0 = ch * CHW                  # within-batch t offset
                    # ---- conv: xc[ct] = depthwise causal conv ----
                    xc = []
                    for ct in range(CT):
                        xt = xtp.tile([128, CHW + 2], BF16, tag="xt")
                        if ch == 0:
                            nc.gpsimd.memset(xt[:, 0:2], 0.0)
                            nc.sync.dma_start(xt[:, 2:CHW + 2], xg[ct * 128:(ct + 1) * 128, g0:g0 + CHW])
                        else:
                            nc.sync.dma_start(xt[:], xg[ct * 128:(ct + 1) * 128, g0 - 2:g0 + CHW])
                        ta = tmp.tile([128, CHW], F32, tag="t1")
                        nc.scalar.mul(ta[:], xt[:, 0:CHW], cw[ct][:, 0:1])
                        tb = tmp.tile([128, CHW], F32, tag="t2")
                        nc.vector.scalar_tensor_tensor(tb[:], xt[:, 1:CHW + 1], cw[ct][:, 1:2], ta[:], OP.mult, OP.add)
                        xct = xcp.tile([128, CHW], BF16, tag=f"xc{ct}")
                        nc.vector.scalar_tensor_tensor(xct[:], xt[:, 2:CHW + 2], cw[ct][:, 2:3], tb[:], OP.mult, OP.add)
                        xc.append(xct)

                    # ---- QKV + rope ----
                    q_sb = []
                    for h in range(HPC):
                        hs = slice(h * D, (h + 1) * D)
                        # q
                        q_ps = pmm.tile([128, CHW], F32, tag="mm")
                        for ct in range(CT):
                            nc.tensor.matmul(q_ps[:], wq_r[ct][:, hs], xc[ct][:],
                                             start=(ct == 0), stop=(ct == CT - 1))
                        a = tmp.tile([128, CHW], F32, tag="t1")
                        nc.vector.tensor_tensor(a[:], q_ps[:], cosT[:, t0:t0 + CHW], OP.mult)
                        bb = tmp.tile([128, CHW], F32, tag="t2")
                        nc.vector.tensor_tensor(bb[0:64, :], q_ps[64:128, :], sinS[0:64, t0:t0 + CHW], OP.mult)
                        nc.vector.tensor_tensor(bb[64:128, :], q_ps[0:64, :], sinS[64:128, t0:t0 + CHW], OP.mult)
                        qt = qsp.tile([128, CHW], BF16, tag=f"q{h}")
                        nc.vector.tensor_tensor(qt[:], a[:], bb[:], OP.add)
                        q_sb.append(qt)
                        # k
                        k_ps = pmm.tile([128, CHW], F32, tag="mm")
                        for ct in range(CT):
                            nc.tensor.matmul(k_ps[:], wk_r[ct][:, hs], xc[ct][:],
                                             start=(ct == 0), stop=(ct == CT - 1))
                        a2 = tmp.tile([128, CHW], F32, tag="t1")
                        nc.vector.tensor_tensor(a2[:], k_ps[:], cosT[:, t0:t0 + CHW], OP.mult)
                        b2 = tmp.tile([128, CHW], F32, tag="t2")
                        nc.vector.tensor_tensor(b2[0:64, :], k_ps[64:128, :], sinS[0:64, t0:t0 + CHW], OP.mult)
                        nc.vector.tensor_tensor(b2[64:128, :], k_ps[0:64, :], sinS[64:128, t0:t0 + CHW], OP.mult)
                        nc.vector.tensor_tensor(k_all[h][:, t0:t0 + CHW], a2[:], b2[:], OP.add)
                    # ---- v: computed directly in [s, d] block layout (both heads) ----
                    for j in range(CHW // 128):
                        v_ps = pV.tile([128, HPC * D], F32, tag="vmm")
                        for ct in range(CT):
                            nc.tensor.matmul(v_ps[:], xc[ct][:, j * 128:(j + 1) * 128], wv_r[ct][:],
                                             start=(ct == 0), stop=(ct == CT - 1))
                        srow = t0 + j * 128
                        for h in range(HPC):
                            nc.vector.tensor_copy(v_all[h][:, srow:srow + 128], v_ps[:, h * D:(h + 1) * D])

                    # ---- attention ----
                    yT = []
                    n_st = 4 * ch + 4
                    for h in range(HPC):
                        U_ps = pU.tile([128, CHW], F32, tag="U")
                        D_ps = pD.tile([128, CHW], F32, tag="Dn")
                        for st in range(n_st):
                            s_ps = pS.tile([128, CHW], F32, tag="S")
                            nc.tensor.matmul(s_ps[:], k_all[h][:, st * STW:(st + 1) * STW], q_sb[h][:],
                                             start=True, stop=True)
                            e = ep.tile([128, CHW], BF16, tag="e")
                            nc.scalar.activation(e[:], s_ps[:], AF.Exp)
                            if st >= 4 * ch:
                                i = st - 4 * ch
                                nc.vector.tensor_tensor(e[:], e[:], masks[:, i * CHW:(i + 1) * CHW], OP.mult)
                            nc.tensor.matmul(U_ps[:], v_all[h][:, st * STW:(st + 1) * STW], e[:],
                                             start=(st == 0), stop=(st == n_st - 1))
                            nc.tensor.matmul(D_ps[:], ones_r[:], e[:],
                                             start=(st == 0), stop=(st == n_st - 1))
                        rD = rdp.tile([128, CHW], F32, tag="rd")
                        nc.vector.reciprocal(rD[:], D_ps[:])
                        yt = yp.tile([128, CHW], BF16, tag=f"y{h}")
                        nc.vector.tensor_tensor(yt[:], U_ps[:], rD[:], OP.mult)
                        yT.append(yt)

                    # ---- proj (partial over this core's heads) ----
                    for oc in range(CT):
                        o_ps = pmm.tile([128, CHW], F32, tag="mm")
                        nc.tensor.matmul(o_ps[:], wo_r[0][:, oc * 128:(oc + 1) * 128], yT[0][:],
                                         start=True, stop=False)
                        nc.tensor.matmul(o_ps[:], wo_r[1][:, oc * 128:(oc + 1) * 128], yT[1][:],
                                         start=False, stop=True)
                        o_sb = vstgp.tile([128, CHW], BF16, tag="osb")
                        nc.scalar.copy(o_sb[:], o_ps[:])
                        nc.sync.dma_start(oacc[oc * 128:(oc + 1) * 128, g0:g0 + CHW], o_sb[:])

            # ---- sum the 8 partial projections; each core keeps its row-slice ----
            nc.gpsimd.collective_compute(
                "ReduceScatter", mybir.AluOpType.add,
                replica_groups=groups, ins=[oacc.opt()], outs=[ors.opt()])
            nc.gpsimd.dma_start(outS[:], ors[:])

    nc.compile()
    return nc


def host_prepare(x, conv_w, w_attn, w_proj):
    """Build per-core input maps."""
    xT = np.ascontiguousarray(x.transpose(2, 0, 1).reshape(C, B * T)).astype(NPBF16)
    convw = np.ascontiguousarray(conv_w[:, 0, :]).astype(np.float32)

    scale = 1.0 / np.sqrt(np.float32(D))
    in_maps = []
    for c in range(NC):
        h0 = c * HPC
        rq = slice(h0 * D, (h0 + HPC) * D)
        wq_c = np.ascontiguousarray((w_attn[rq.start:rq.stop, :] * scale).T).astype(NPBF16)
        wk_c = np.ascontiguousarray(w_attn[C + rq.start:C + rq.stop, :].T).astype(NPBF16)
        wv_c = np.ascontiguousarray(w_attn[2 * C + rq.start:2 * C + rq.stop, :].T).astype(NPBF16)
        wo_c = np.ascontiguousarray(w_proj[:, rq.start:rq.stop].T).astype(NPBF16)
        xs_c = np.ascontiguousarray(xT[c * CSH:(c + 1) * CSH, :])
        in_maps.append({
            "xs": xs_c, "wq": wq_c, "wk": wk_c, "wv": wv_c, "wo": wo_c,
            "convw": convw,
        })
    return in_maps


def host_finish(results):
    full = np.concatenate([np.asarray(r["outS"], dtype=np.float32) for r in results], axis=0)   # [C, B*T]
    return full.reshape(C, B, T).transpose(1, 2, 0).astype(np.float32)


_CACHE = {}


def kernel(x, conv_w, w_attn, w_proj):
    x = np.ascontiguousarray(x, dtype=np.float32)
    conv_w = np.ascontiguousarray(conv_w, dtype=np.float32)
    w_attn = np.ascontiguousarray(w_attn, dtype=np.float32)
    w_proj = np.ascontiguousarray(w_proj, dtype=np.float32)
    if "nc" not in _CACHE:
        _CACHE["nc"] = build_program()
    in_maps = host_prepare(x, conv_w, w_attn, w_proj)
    res = bass_utils.run_bass_kernel_spmd(_CACHE["nc"], in_maps, core_ids=list(range(NC)))
    return host_finish(res.results)
